# revision 41
# baseline (speedup 1.0000x reference)
"""Trainium2 Bass kernel for DilateAttention (3x3 kernel, dilation 2).

q,k,v: [B=4, d=384, H=64, W=64] f32.  heads=12, head_dim=32.
out: [B, H, W, d] f32.  Core = (batch b, row-half); 8 cores.

v2 design (vs baseline):
  - logits: merged M=36 matmuls (contiguous logit rows 0..35, no memset)
  - softmax pre-normalized: Z replicated to 36 rows via selz matmul,
    1/Z on DVE, attn = exp * zrec (bf16) BEFORE broadcast
  - bc: K=36 matmuls -> PSUM f32, ACT copies -> bf16 SBUF
  - pav: DVE bf16 2x mults (3-dj batched)
  - ksum: chunks 0-1 on PE (identity accumulate + ACT copy),
    chunks 2-3 on DVE add tree -> acc bf16
  - products: di 0,1 + (1,h0) on DVE, rest on GPSIMD
  - output: channel-major bf16 [384, 2048] per core; host transposes/casts
"""

import os
import sys

for _p in ("/opt/trn_rl_repo", "/root/.axon_site/_ro/trn_rl_repo"):
    if _p not in sys.path and os.path.isdir(_p):
        sys.path.insert(0, _p)

import dataclasses
from contextlib import ExitStack

import numpy as np
import ml_dtypes

import concourse.bass as bass
import concourse.bacc as bacc
import concourse.mybir as mybir
import concourse.tile as tile
from concourse import masks
from concourse.bass_utils import run_bass_kernel_spmd

BF16 = ml_dtypes.bfloat16

B, D, H, W = 4, 384, 64, 64
NH, HD = 12, 32
KK, DIL, PAD = 3, 2, 2
K2 = KK * KK
SCALE = HD ** -0.5

NCORES = 8
ROWS = H // 2              # 32 output rows per core
HROWS = ROWS + 2 * PAD     # 36 halo rows of padded k/v
WP = W + 2 * PAD           # 68 padded width
NGRP = 3                   # channel groups of 128 partitions
HPG = 4                    # heads per group
FD = ROWS * W              # 2048 pixels per core
NCH = 4                    # chunks
CHD = FD // NCH            # 512 chunk free dim
LG = HPG * K2              # 36 logit rows per group

_CACHE = {}


def _build_sel_constants():
    # logits reduce: for offset ko, lhsT[p=hl*32+c, m=hl*9+ko] = 1 (M=36)
    selqk = np.zeros((128, K2, LG), np.float32)
    for hl in range(HPG):
        for c in range(HD):
            for ko in range(K2):
                selqk[hl * HD + c, ko, hl * K2 + ko] = 1.0
    # Z: zp[hl, l] = sum_ko exp[hl*9+ko, l]
    selz = np.zeros((LG, HPG), np.float32)
    for hl in range(HPG):
        for ko in range(K2):
            selz[hl * K2 + ko, hl] = 1.0
    # bc: for offset ko, lhsT[p=hl*9+ko, m=hl*32+c] = 1 (K=36, M=128)
    selbc = np.zeros((LG, K2, 128), np.float32)
    for hl in range(HPG):
        for ko in range(K2):
            for c in range(HD):
                selbc[hl * K2 + ko, ko, hl * HD + c] = 1.0
    return (
        selqk.reshape(128, K2 * LG).astype(BF16),
        selz.astype(BF16),
        selbc.reshape(LG, K2 * 128).astype(BF16),
    )


def _build_nc():
    nc = bacc.Bacc("TRN2", target_bir_lowering=False, debug=False,
                   num_devices=NCORES)
    f32 = mybir.dt.float32
    bf16 = mybir.dt.bfloat16

    q_p = nc.declare_dram_parameter("q", [D, FD], bf16, isOutput=False)
    k_p = nc.declare_dram_parameter("k", [D, HROWS * WP], bf16, isOutput=False)
    v_p = nc.declare_dram_parameter("v", [D, HROWS * WP], bf16, isOutput=False)
    selqk_p = nc.declare_dram_parameter("selqk", [128, K2 * LG], bf16, isOutput=False)
    selz_p = nc.declare_dram_parameter("selz", [LG, HPG], bf16, isOutput=False)
    selbc_p = nc.declare_dram_parameter("selbc", [LG, K2 * 128], bf16, isOutput=False)
    out_p = nc.declare_dram_parameter("out", [D, FD], bf16, isOutput=True)
    z_p = nc.declare_dram_parameter("z", [NGRP * HPG, FD], f32, isOutput=True)

    with tile.TileContext(nc) as tc, ExitStack() as ctx:
        consts = ctx.enter_context(tc.tile_pool(name="consts", bufs=1))
        inp = ctx.enter_context(tc.tile_pool(name="inp", bufs=3))
        prods = ctx.enter_context(tc.tile_pool(name="prods", bufs=2))
        smax = ctx.enter_context(tc.tile_pool(name="smax", bufs=3))
        accp = ctx.enter_context(tc.tile_pool(name="accp", bufs=2))
        # PSUM: lg/zp share (bufs=3), bc (bufs=4), ksum acc (bufs=1)
        ps_sm = ctx.enter_context(tc.tile_pool(name="ps_sm", bufs=2, space="PSUM"))
        ps_bc = ctx.enter_context(tc.tile_pool(name="ps_bc", bufs=2, space="PSUM"))

        selqk_t = consts.tile([128, K2 * LG], bf16)
        nc.sync.dma_start(selqk_t[:], selqk_p[:])
        selz_t = consts.tile([LG, HPG], bf16)
        nc.sync.dma_start(selz_t[:], selz_p[:])
        selbc_t = consts.tile([LG, K2 * 128], bf16)
        nc.sync.dma_start(selbc_t[:], selbc_p[:])
        ident = consts.tile([128, 128], bf16)
        masks.make_identity(nc, ident[:])

        KSPLIT = 20 * WP

        def load_group(g):
            gp = slice(g * 128, (g + 1) * 128)
            q_t = inp.tile([128, FD], bf16, tag="q", name="q_t")
            k_t = inp.tile([128, HROWS * WP], bf16, tag="k", name="k_t")
            nc.sync.dma_start(q_t[:, :FD // 2], q_p[gp, :FD // 2])
            nc.sync.dma_start(k_t[:, :KSPLIT], k_p[gp, :KSPLIT])
            nc.sync.dma_start(q_t[:, FD // 2:], q_p[gp, FD // 2:])
            nc.sync.dma_start(k_t[:, KSPLIT:], k_p[gp, KSPLIT:])
            v_t = inp.tile([128, HROWS * WP], bf16, tag="v", name="v_t")
            nc.sync.dma_start(v_t[:, :KSPLIT], v_p[gp, :KSPLIT])
            nc.sync.dma_start(v_t[:, KSPLIT:], v_p[gp, KSPLIT:])
            k3 = k_t[:].rearrange("p (r w) -> p r w", r=HROWS)
            v3 = v_t[:].rearrange("p (r w) -> p r w", r=HROWS)
            return q_t, k3, v3

        def dj_triple(t3, di, r0, nr):
            sl = t3[:, DIL * di + r0: DIL * di + r0 + nr, 0:W]
            return dataclasses.replace(
                sl, ap=[sl.ap[0], [DIL, KK]] + list(sl.ap[1:]))

        def rep3(flat, nr):
            sl = flat.rearrange("p (r w) -> p r w", r=nr)
            return dataclasses.replace(
                sl, ap=[sl.ap[0], [0, KK]] + list(sl.ap[1:]))

        QR = ROWS // NCH          # 8 rows per quarter

        def alloc_prods():
            pts = [prods.tile([128, KK * FD], bf16, tag=f"pd{di}",
                              name=f"pd{di}") for di in range(KK)]
            ptiles = []
            for di in range(KK):
                ptiles.extend(pts[di][:, dj * FD:(dj + 1) * FD]
                              for dj in range(KK))
            return pts, ptiles

        def emit_prods_q(q_t, k3, pts, q, gps=False):
            # products for row-quarter q (pixels q*512..q*512+512), all 9 ko
            for di in range(KK):
                pt4 = pts[di][:].rearrange("p (k r w) -> p k r w",
                                           k=KK, r=ROWS)
                nc.vector.tensor_mul(
                    pt4[:, :, q * QR:(q + 1) * QR, :],
                    rep3(q_t[:, q * QR * W:(q + 1) * QR * W], QR),
                    dj_triple(k3, di, q * QR, QR),
                )

        def emit_softmax_chunk(ptiles, exp_t, zsb, ch):
            # logits (9 acc MMs) -> exp -> Z (normalization on host)
            cs = slice(ch * CHD, (ch + 1) * CHD)
            lg = ps_sm.tile([LG, CHD], f32, tag="sm", name="lg")
            for ko in range(K2):
                nc.tensor.matmul(
                    lg[:], selqk_t[:, ko * LG:(ko + 1) * LG],
                    ptiles[ko][:, cs],
                    start=(ko == 0), stop=(ko == K2 - 1),
                )
            nc.scalar.activation(
                exp_t[:, cs], lg[:],
                mybir.ActivationFunctionType.Exp, scale=float(SCALE),
            )
            zp = ps_sm.tile([HPG, CHD], f32, tag="sm", name="zp")
            nc.tensor.matmul(zp[:], selz_t[:], exp_t[:, cs],
                             start=True, stop=True)
            nc.scalar.copy(zsb[:, cs], zp[:])

        def emit_bc_pav(exp_t, v3, pc):
            cs = slice(pc * CHD, (pc + 1) * CHD)
            r0 = pc * (CHD // W)
            nr = CHD // W
            pavs = []
            for di in range(KK):
                bcs3 = smax.tile([128, KK * CHD], bf16, tag="bcs3",
                                 name="bcs3")
                bc3 = ps_bc.tile([128, KK * CHD], f32, tag="bc", name="bc3")
                for dj in range(KK):
                    ko = di * KK + dj
                    nc.tensor.matmul(
                        bc3[:, dj * CHD:(dj + 1) * CHD],
                        selbc_t[:, ko * 128:(ko + 1) * 128],
                        exp_t[:, cs], start=True, stop=True,
                    )
                nc.scalar.copy(bcs3[:], bc3[:])
                pav = accp.tile([128, KK * CHD], bf16, tag=f"pav{di}",
                                name=f"pav{di}")
                nc.vector.tensor_mul(
                    pav[:].rearrange("p (k r w) -> p k r w", k=KK, r=nr),
                    bcs3[:].rearrange("p (k r w) -> p k r w", k=KK, r=nr),
                    dj_triple(v3, di, r0, nr),
                )
                pavs.append(pav)
            return pavs

        def emit_ksum(acc_t, pc, pavs):
            cs = slice(pc * CHD, (pc + 1) * CHD)
            if pc % 2 == 0:
                # PE identity-accumulate (acc shares the ps_bc rotation)
                acc_ps = ps_bc.tile([128, CHD], f32, tag="bc", name="acc_ps")
                ko = 0
                for di in range(KK):
                    for dj in range(KK):
                        nc.tensor.matmul(
                            acc_ps[:], ident[:],
                            pavs[di][:, dj * CHD:(dj + 1) * CHD],
                            start=(ko == 0), stop=(ko == K2 - 1))
                        ko += 1
                nc.scalar.copy(acc_t[:, cs], acc_ps[:])
            else:
                # DVE add tree with wide [128, 3*CHD] first level:
                # u = pav_d0 + pav_d1 + pav_d2 (dj-partials), then fold dj
                u = smax.tile([128, KK * CHD], bf16, tag="ks0", name="u")
                s01 = smax.tile([128, CHD], bf16, tag="ks1", name="s01")
                nc.vector.tensor_add(u[:], pavs[0][:], pavs[1][:])
                nc.vector.tensor_add(u[:], u[:], pavs[2][:])
                nc.vector.tensor_add(s01[:], u[:, :CHD], u[:, CHD:2 * CHD])
                nc.vector.tensor_add(acc_t[:, cs], s01[:],
                                     u[:, 2 * CHD:])

        # software pipeline: per group, quartered softmax feeds the AV
        # pc-loop; next group's products interleave into the AV phases
        # (one quarter per phase, di=2 on GPSIMD) so the in-order DVE
        # queue never head-of-line blocks.
        tiles = load_group(0)
        pts, ptiles = alloc_prods()
        for q in range(NCH):
            emit_prods_q(tiles[0], tiles[1], pts, q)
        for g in range(NGRP):
            v3_cur = tiles[2]
            exp_t = smax.tile([LG, FD], bf16, tag="exp", name="exp_t")
            zsb = smax.tile([HPG, FD], f32, tag="zsb", name="zsb")
            for ch in range(NCH):
                emit_softmax_chunk(ptiles, exp_t, zsb, ch)
            nc.sync.dma_start(z_p[g * HPG:(g + 1) * HPG, :], zsb[:])
            if g + 1 < NGRP:
                tiles = load_group(g + 1)
                pts, ptiles = alloc_prods()
            acc_t = smax.tile([128, FD], bf16, tag="acc", name="acc_t")
            prev = None
            for pc in range(NCH):
                pavs = emit_bc_pav(exp_t, v3_cur, pc)
                if g + 1 < NGRP:
                    emit_prods_q(tiles[0], tiles[1], pts, pc)
                if prev is not None:
                    emit_ksum(acc_t, prev[0], prev[1])
                prev = (pc, pavs)
            emit_ksum(acc_t, prev[0], prev[1])
            nc.sync.dma_start(out_p[g * 128:(g + 1) * 128, :], acc_t[:])

    nc.compile()
    return nc


def _get_nc():
    if "nc" not in _CACHE:
        _CACHE["nc"] = _build_nc()
    return _CACHE["nc"]


def build_in_maps(q, k, v):
    qb = np.asarray(q, np.float32).astype(BF16)
    kp = np.pad(np.asarray(k, np.float32),
                ((0, 0), (0, 0), (PAD, PAD), (PAD, PAD))).astype(BF16)
    vp = np.pad(np.asarray(v, np.float32),
                ((0, 0), (0, 0), (PAD, PAD), (PAD, PAD))).astype(BF16)
    selqk, selz, selbc = _CACHE.setdefault("sel", _build_sel_constants())
    in_maps = []
    for c in range(NCORES):
        b, half = divmod(c, 2)
        r0 = half * ROWS
        in_maps.append({
            "q": np.ascontiguousarray(qb[b, :, r0:r0 + ROWS, :]).reshape(D, FD),
            "k": np.ascontiguousarray(kp[b, :, r0:r0 + HROWS, :]).reshape(D, HROWS * WP),
            "v": np.ascontiguousarray(vp[b, :, r0:r0 + HROWS, :]).reshape(D, HROWS * WP),
            "selqk": selqk, "selz": selz, "selbc": selbc,
        })
    return in_maps


def kernel(q, k, v):
    in_maps = build_in_maps(q, k, v)
    nc = _get_nc()
    res = run_bass_kernel_spmd(nc, in_maps, core_ids=list(range(NCORES)))
    out = np.empty((B, H, W, D), np.float32)
    for c in range(NCORES):
        b, half = divmod(c, 2)
        r0 = half * ROWS
        # out dram is [D, FD] channel-major UNNORMALIZED bf16; z is [12, FD]
        ocm = res.results[c]["out"].astype(np.float32)
        z = res.results[c]["z"]                       # [12, FD] f32
        ocm /= np.repeat(z, HD, axis=0)               # [384, FD]
        out[b, r0:r0 + ROWS] = ocm.T.reshape(ROWS, W, D)
    return out


# revision 43
# speedup vs baseline: 1.0985x; 1.0985x over previous
"""Trainium2 Bass kernel for DilateAttention (3x3 kernel, dilation 2).

q,k,v: [B=4, d=384, H=64, W=64] f32.  heads=12, head_dim=32.
out: [B, H, W, d] f32.  Core = (batch b, row-half); 8 cores.

v2 design (vs baseline):
  - logits: merged M=36 matmuls (contiguous logit rows 0..35, no memset)
  - softmax pre-normalized: Z replicated to 36 rows via selz matmul,
    1/Z on DVE, attn = exp * zrec (bf16) BEFORE broadcast
  - bc: K=36 matmuls -> PSUM f32, ACT copies -> bf16 SBUF
  - pav: DVE bf16 2x mults (3-dj batched)
  - ksum: chunks 0-1 on PE (identity accumulate + ACT copy),
    chunks 2-3 on DVE add tree -> acc bf16
  - products: di 0,1 + (1,h0) on DVE, rest on GPSIMD
  - output: channel-major bf16 [384, 2048] per core; host transposes/casts
"""

import os
import sys

for _p in ("/opt/trn_rl_repo", "/root/.axon_site/_ro/trn_rl_repo"):
    if _p not in sys.path and os.path.isdir(_p):
        sys.path.insert(0, _p)

import dataclasses
from contextlib import ExitStack

import numpy as np
import ml_dtypes

import concourse.bass as bass
import concourse.bacc as bacc
import concourse.mybir as mybir
import concourse.tile as tile
from concourse import masks
from concourse.bass_utils import run_bass_kernel_spmd

BF16 = ml_dtypes.bfloat16

B, D, H, W = 4, 384, 64, 64
NH, HD = 12, 32
KK, DIL, PAD = 3, 2, 2
K2 = KK * KK
SCALE = HD ** -0.5

NCORES = 8
ROWS = H // 2              # 32 output rows per core
HROWS = ROWS + 2 * PAD     # 36 halo rows of padded k/v
WP = W + 2 * PAD           # 68 padded width
NGRP = 3                   # channel groups of 128 partitions
HPG = 4                    # heads per group
FD = ROWS * W              # 2048 pixels per core
NCH = 4                    # chunks
CHD = FD // NCH            # 512 chunk free dim
LG = HPG * K2              # 36 logit rows per group

_CACHE = {}


def _build_sel_constants():
    # logits reduce: for offset ko, lhsT[p=hl*32+c, m=hl*9+ko] = 1 (M=36)
    selqk = np.zeros((128, K2, LG), np.float32)
    for hl in range(HPG):
        for c in range(HD):
            for ko in range(K2):
                selqk[hl * HD + c, ko, hl * K2 + ko] = 1.0
    # Z: zp[hl, l] = sum_ko exp[hl*9+ko, l]
    selz = np.zeros((LG, HPG), np.float32)
    for hl in range(HPG):
        for ko in range(K2):
            selz[hl * K2 + ko, hl] = 1.0
    # bc: for offset ko, lhsT[p=hl*9+ko, m=hl*32+c] = 1 (K=36, M=128)
    selbc = np.zeros((LG, K2, 128), np.float32)
    for hl in range(HPG):
        for ko in range(K2):
            for c in range(HD):
                selbc[hl * K2 + ko, ko, hl * HD + c] = 1.0
    return (
        selqk.reshape(128, K2 * LG).astype(BF16),
        selz.astype(BF16),
        selbc.reshape(LG, K2 * 128).astype(BF16),
    )


def _build_nc():
    nc = bacc.Bacc("TRN2", target_bir_lowering=False, debug=False,
                   num_devices=NCORES)
    f32 = mybir.dt.float32
    bf16 = mybir.dt.bfloat16

    q_p = nc.declare_dram_parameter("q", [D, FD], bf16, isOutput=False)
    k_p = nc.declare_dram_parameter("k", [D, HROWS * WP], bf16, isOutput=False)
    v_p = nc.declare_dram_parameter("v", [D, HROWS * WP], bf16, isOutput=False)
    selqk_p = nc.declare_dram_parameter("selqk", [128, K2 * LG], bf16, isOutput=False)
    selz_p = nc.declare_dram_parameter("selz", [LG, HPG], bf16, isOutput=False)
    selbc_p = nc.declare_dram_parameter("selbc", [LG, K2 * 128], bf16, isOutput=False)
    out_p = nc.declare_dram_parameter("out", [D, FD], bf16, isOutput=True)
    z_p = nc.declare_dram_parameter("z", [NGRP * HPG, FD], f32, isOutput=True)

    with tile.TileContext(nc) as tc, ExitStack() as ctx:
        consts = ctx.enter_context(tc.tile_pool(name="consts", bufs=1))
        inp = ctx.enter_context(tc.tile_pool(name="inp", bufs=3))
        prods = ctx.enter_context(tc.tile_pool(name="prods", bufs=2))
        smax = ctx.enter_context(tc.tile_pool(name="smax", bufs=3))
        accp = ctx.enter_context(tc.tile_pool(name="accp", bufs=2))
        # PSUM: lg/zp share (bufs=3), bc (bufs=4), ksum acc (bufs=1)
        ps_sm = ctx.enter_context(tc.tile_pool(name="ps_sm", bufs=2, space="PSUM"))
        ps_bc = ctx.enter_context(tc.tile_pool(name="ps_bc", bufs=2, space="PSUM"))

        selqk_t = consts.tile([128, K2 * LG], bf16)
        nc.sync.dma_start(selqk_t[:], selqk_p[:])
        selz_t = consts.tile([LG, HPG], bf16)
        nc.sync.dma_start(selz_t[:], selz_p[:])
        selbc_t = consts.tile([LG, K2 * 128], bf16)
        nc.sync.dma_start(selbc_t[:], selbc_p[:])
        ident = consts.tile([128, 128], bf16)
        masks.make_identity(nc, ident[:])

        KSPLIT = 20 * WP

        def load_group(g):
            gp = slice(g * 128, (g + 1) * 128)
            q_t = inp.tile([128, FD], bf16, tag="q", name="q_t")
            k_t = inp.tile([128, HROWS * WP], bf16, tag="k", name="k_t")
            nc.sync.dma_start(q_t[:, :FD // 2], q_p[gp, :FD // 2])
            nc.sync.dma_start(k_t[:, :KSPLIT], k_p[gp, :KSPLIT])
            nc.sync.dma_start(q_t[:, FD // 2:], q_p[gp, FD // 2:])
            nc.sync.dma_start(k_t[:, KSPLIT:], k_p[gp, KSPLIT:])
            v_t = inp.tile([128, HROWS * WP], bf16, tag="v", name="v_t")
            nc.sync.dma_start(v_t[:, :KSPLIT], v_p[gp, :KSPLIT])
            nc.sync.dma_start(v_t[:, KSPLIT:], v_p[gp, KSPLIT:])
            k3 = k_t[:].rearrange("p (r w) -> p r w", r=HROWS)
            v3 = v_t[:].rearrange("p (r w) -> p r w", r=HROWS)
            return q_t, k3, v3

        def dj_triple(t3, di, r0, nr):
            sl = t3[:, DIL * di + r0: DIL * di + r0 + nr, 0:W]
            return dataclasses.replace(
                sl, ap=[sl.ap[0], [DIL, KK]] + list(sl.ap[1:]))

        def rep3(flat, nr):
            sl = flat.rearrange("p (r w) -> p r w", r=nr)
            return dataclasses.replace(
                sl, ap=[sl.ap[0], [0, KK]] + list(sl.ap[1:]))

        QR = ROWS // NCH          # 8 rows per quarter

        def alloc_prods():
            pts = [prods.tile([128, KK * FD], bf16, tag=f"pd{di}",
                              name=f"pd{di}") for di in range(KK)]
            ptiles = []
            for di in range(KK):
                ptiles.extend(pts[di][:, dj * FD:(dj + 1) * FD]
                              for dj in range(KK))
            return pts, ptiles

        def emit_prods_q(q_t, k3, pts, q, gps=False):
            # products for row-quarter q (pixels q*512..q*512+512), all 9 ko
            for di in range(KK):
                pt4 = pts[di][:].rearrange("p (k r w) -> p k r w",
                                           k=KK, r=ROWS)
                nc.vector.tensor_mul(
                    pt4[:, :, q * QR:(q + 1) * QR, :],
                    rep3(q_t[:, q * QR * W:(q + 1) * QR * W], QR),
                    dj_triple(k3, di, q * QR, QR),
                )

        def emit_softmax_chunk(ptiles, exp_t, zsb, ch):
            # logits (9 acc MMs) -> exp -> Z (normalization on host)
            cs = slice(ch * CHD, (ch + 1) * CHD)
            lg = ps_sm.tile([LG, CHD], f32, tag="sm", name="lg")
            for ko in range(K2):
                nc.tensor.matmul(
                    lg[:], selqk_t[:, ko * LG:(ko + 1) * LG],
                    ptiles[ko][:, cs],
                    start=(ko == 0), stop=(ko == K2 - 1),
                )
            nc.scalar.activation(
                exp_t[:, cs], lg[:],
                mybir.ActivationFunctionType.Exp, scale=float(SCALE),
            )
            zp = ps_sm.tile([HPG, CHD], f32, tag="sm", name="zp")
            nc.tensor.matmul(zp[:], selz_t[:], exp_t[:, cs],
                             start=True, stop=True)
            nc.scalar.copy(zsb[:, cs], zp[:])

        def emit_bc_pav(exp_t, v3, pc):
            cs = slice(pc * CHD, (pc + 1) * CHD)
            r0 = pc * (CHD // W)
            nr = CHD // W
            pavs = []
            for di in range(KK):
                bcs3 = smax.tile([128, KK * CHD], bf16, tag="bcs3",
                                 name="bcs3")
                bc3 = ps_bc.tile([128, KK * CHD], f32, tag="bc", name="bc3")
                for dj in range(KK):
                    ko = di * KK + dj
                    nc.tensor.matmul(
                        bc3[:, dj * CHD:(dj + 1) * CHD],
                        selbc_t[:, ko * 128:(ko + 1) * 128],
                        exp_t[:, cs], start=True, stop=True,
                    )
                nc.scalar.copy(bcs3[:], bc3[:])
                pav = accp.tile([128, KK * CHD], bf16, tag=f"pav{di}",
                                name=f"pav{di}")
                nc.vector.tensor_mul(
                    pav[:].rearrange("p (k r w) -> p k r w", k=KK, r=nr),
                    bcs3[:].rearrange("p (k r w) -> p k r w", k=KK, r=nr),
                    dj_triple(v3, di, r0, nr),
                )
                pavs.extend(pav[:, dj * CHD:(dj + 1) * CHD]
                            for dj in range(KK))
            return pavs

        def emit_ksum(acc_t, pc, pavs):
            cs = slice(pc * CHD, (pc + 1) * CHD)
            if pc % 2 == 0:
                # PE identity-accumulate (acc shares the ps_bc rotation)
                acc_ps = ps_bc.tile([128, CHD], f32, tag="bc", name="acc_ps")
                for ko in range(K2):
                    nc.tensor.matmul(acc_ps[:], ident[:], pavs[ko],
                                     start=(ko == 0), stop=(ko == K2 - 1))
                nc.scalar.copy(acc_t[:, cs], acc_ps[:])
            else:
                # DVE add tree
                s01 = smax.tile([128, CHD], bf16, tag="ks0", name="s01")
                s23 = smax.tile([128, CHD], bf16, tag="ks1", name="s23")
                s45 = smax.tile([128, CHD], bf16, tag="ks2", name="s45")
                s67 = smax.tile([128, CHD], bf16, tag="ks3", name="s67")
                nc.vector.tensor_add(s01[:], pavs[0], pavs[1])
                nc.vector.tensor_add(s23[:], pavs[2], pavs[3])
                nc.vector.tensor_add(s45[:], pavs[4], pavs[5])
                nc.vector.tensor_add(s67[:], pavs[6], pavs[7])
                nc.vector.tensor_add(s01[:], s01[:], s23[:])
                nc.vector.tensor_add(s45[:], s45[:], s67[:])
                nc.vector.tensor_add(s01[:], s01[:], s45[:])
                nc.vector.tensor_add(acc_t[:, cs], s01[:], pavs[8])

        # software pipeline: per group, quartered softmax feeds the AV
        # pc-loop; next group's products interleave into the AV phases
        # (one quarter per phase, di=2 on GPSIMD) so the in-order DVE
        # queue never head-of-line blocks.
        tiles = load_group(0)
        pts, ptiles = alloc_prods()
        for q in range(NCH):
            emit_prods_q(tiles[0], tiles[1], pts, q)
        for g in range(NGRP):
            v3_cur = tiles[2]
            exp_t = smax.tile([LG, FD], bf16, tag="exp", name="exp_t")
            zsb = smax.tile([HPG, FD], f32, tag="zsb", name="zsb")
            for ch in range(NCH):
                emit_softmax_chunk(ptiles, exp_t, zsb, ch)
            nc.sync.dma_start(z_p[g * HPG:(g + 1) * HPG, :], zsb[:])
            if g + 1 < NGRP:
                tiles = load_group(g + 1)
                pts, ptiles = alloc_prods()
            acc_t = smax.tile([128, FD], bf16, tag="acc", name="acc_t")
            prev = None
            for pc in range(NCH):
                pavs = emit_bc_pav(exp_t, v3_cur, pc)
                if g + 1 < NGRP:
                    emit_prods_q(tiles[0], tiles[1], pts, pc)
                if prev is not None:
                    emit_ksum(acc_t, prev[0], prev[1])
                prev = (pc, pavs)
            emit_ksum(acc_t, prev[0], prev[1])
            nc.sync.dma_start(out_p[g * 128:(g + 1) * 128, :], acc_t[:])

    nc.compile()
    return nc


def _get_nc():
    if "nc" not in _CACHE:
        _CACHE["nc"] = _build_nc()
    return _CACHE["nc"]


def build_in_maps(q, k, v):
    qb = np.asarray(q, np.float32).astype(BF16)
    kp = np.pad(np.asarray(k, np.float32),
                ((0, 0), (0, 0), (PAD, PAD), (PAD, PAD))).astype(BF16)
    vp = np.pad(np.asarray(v, np.float32),
                ((0, 0), (0, 0), (PAD, PAD), (PAD, PAD))).astype(BF16)
    selqk, selz, selbc = _CACHE.setdefault("sel", _build_sel_constants())
    in_maps = []
    for c in range(NCORES):
        b, half = divmod(c, 2)
        r0 = half * ROWS
        in_maps.append({
            "q": np.ascontiguousarray(qb[b, :, r0:r0 + ROWS, :]).reshape(D, FD),
            "k": np.ascontiguousarray(kp[b, :, r0:r0 + HROWS, :]).reshape(D, HROWS * WP),
            "v": np.ascontiguousarray(vp[b, :, r0:r0 + HROWS, :]).reshape(D, HROWS * WP),
            "selqk": selqk, "selz": selz, "selbc": selbc,
        })
    return in_maps


def kernel(q, k, v):
    in_maps = build_in_maps(q, k, v)
    nc = _get_nc()
    res = run_bass_kernel_spmd(nc, in_maps, core_ids=list(range(NCORES)))
    out = np.empty((B, H, W, D), np.float32)
    for c in range(NCORES):
        b, half = divmod(c, 2)
        r0 = half * ROWS
        # out dram is [D, FD] channel-major UNNORMALIZED bf16; z is [12, FD]
        ocm = res.results[c]["out"].astype(np.float32)
        z = res.results[c]["z"]                       # [12, FD] f32
        ocm /= np.repeat(z, HD, axis=0)               # [384, FD]
        out[b, r0:r0 + ROWS] = ocm.T.reshape(ROWS, W, D)
    return out
